# revision 25
# baseline (speedup 1.0000x reference)
"""Causal multi-head attention on 8 Trainium2 NeuronCores.

Problem: x[8,1024,768], 12 heads of d_head=64, causal softmax attention,
output projection. Sharding: data-parallel over batch (8 batch elements ==
8 cores), no collectives.

Per-core layout strategy (zero on-device transposes):
  - host passes xT [768,1024] (d_model on partitions)
  - qT/kT [768,1024] head-major rows  (d_head on partitions, seq on free)
  - scoresT[k, q] = kT_blk.T @ qT     (k on partitions, q on free)
  - v_aug [1024, 12*128]: per head 64 v columns + 64 ones columns; the AV
    matmul zT_psum = v_aug_blk.T @ exp(scoresT) then yields the softmax
    denominator (replicated) in psum partitions 64..127 for free
  - normalize with DVE reciprocal + tensor_mul (one PSUM operand)
  - out = zT.T @ W_O accumulated over head-pair chunks
Causal structure: only the lower-triangular (k <= q) blocks are computed;
the diagonal 128x128 block is zeroed above the diagonal post-exp via
affine_select. exp is computed without max subtraction (scores are O(1)
by construction, exp(-1e5) == 0 in fp32 matches the reference's masking).
All matmuls run as float32r (full PE rate) accumulating fp32 in PSUM.
"""

from contextlib import ExitStack

import numpy as np

import concourse.bass as bass
import concourse.mybir as mybir
import concourse.tile as tile
from concourse import bacc, bass_utils

F32 = mybir.dt.float32
FR = mybir.dt.float32r
BF = mybir.dt.bfloat16
import os
BF16_LHS = os.environ.get("BASS_BF16_LHS", "0") == "1"
WDT = BF if BF16_LHS else FR       # dtype of kT / vaug / wq / wk (lhsT-only)

S = 1024        # seq len
D = 768         # d_model
H = 12          # heads
DH = 64         # d_head
P = 128         # partitions
KC = D // P     # 6 k-chunks of d_model
SB = S // P     # 8 seq blocks
PAIRS = H // 2  # 6 head pairs
VW = 2 * DH     # 128: v cols + ones cols per head in v_aug
N_CORES = 8


def fr(ap):
    return ap


def attention_kernel(tc, out_ap, ins):
    nc = tc.nc
    with ExitStack() as ctx:
        cpool = ctx.enter_context(tc.tile_pool(name="consts", bufs=1))
        big = ctx.enter_context(tc.tile_pool(name="big", bufs=1))
        exp_pool = ctx.enter_context(tc.tile_pool(name="exp", bufs=3))
        rec_pool = ctx.enter_context(tc.tile_pool(name="rec", bufs=2))
        out_pool = ctx.enter_context(tc.tile_pool(name="outb", bufs=3))
        pp = ctx.enter_context(tc.tile_pool(name="ps", bufs=8, space="PSUM"))

        # ---- big persistent tiles.  zT reuses xT's storage: xT is fully
        # consumed by the projections before any zT column is written (the
        # scheduler enforces the WAR ordering via subtile deps).
        xT = big.tile([P, KC * S], FR, tag="xT")          # xT[kc]: cols kc*1024
        qT = big.tile([P, PAIRS * S], FR, tag="qT")       # pair p: cols p*1024+q
        kT = big.tile([P, PAIRS * S], WDT, tag="kT")
        vaug = big.tile([P, SB * H * VW], WDT, tag="vaug")  # blk j: j*1536 + n*128
        zT = xT

        # xT loads issued on the ACT sequencer so they don't queue behind the
        # W loads on SP (each DMA costs ~0.6us of issue time per sequencer)
        for kc in range(KC):
            nc.scalar.dma_start(xT[:, kc * S:(kc + 1) * S],
                                ins["xT"][kc * P:(kc + 1) * P, :])

        # ---- constants / small tiles (issued on the Pool sequencer)
        bqt = cpool.tile([P, PAIRS], F32, tag="bqt")   # b_Q per head-major row
        bkt = cpool.tile([P, PAIRS], F32, tag="bkt")
        bvb = cpool.tile([P, D], F32, tag="bvb")       # b_V broadcast to 128 rows
        bob = cpool.tile([P, D], F32, tag="bob")       # b_O broadcast to 128 rows
        nc.gpsimd.dma_start(bqt[:], ins["bqt"].rearrange("(c p) x -> p (c x)", p=P))
        nc.gpsimd.dma_start(bkt[:], ins["bkt"].rearrange("(c p) x -> p (c x)", p=P))
        nc.gpsimd.dma_start(bvb[:], ins["bvb"][:])
        nc.gpsimd.dma_start(bob[:], ins["bob"][:])

        # ones columns of v_aug (cols 64..127 of each head block), filled by
        # DVE copies (f32 -> fp32r rounding) from a memset source tile
        ones_f = cpool.tile([P, D], F32, tag="ones_f")
        nc.gpsimd.memset(ones_f[:], 1.0)
        ones_f3 = ones_f[:].rearrange("p (x c) -> p x c", c=DH)  # [128, 12, 64]
        for s in range(SB):
            blk = vaug[:, s * H * VW:(s + 1) * H * VW]
            va3 = blk.rearrange("p (x c) -> p x c", c=VW)
            nc.vector.tensor_copy(va3[:, :, DH:VW], ones_f3)

        # per-pair column slices of W_Q/W_K, streamed: [768, 128] -> [128, 6*128]
        wq_r = ins["wq"].rearrange("(c p) n -> p c n", p=P)
        wk_r = ins["wk"].rearrange("(c p) n -> p c n", p=P)

        with tc.tile_pool(name="wa", bufs=1) as wa:
            wv = wa.tile([P, KC * D], FR, tag="wv")
            for kc in range(KC):
                nc.gpsimd.dma_start(wv[:, kc * D:(kc + 1) * D],
                                    ins["wv"][kc * P:(kc + 1) * P, :])

            # ---- Q/K projections: qT[pair rows, q] = W[:, pair].T @ xT
            for p in range(PAIRS):
                wqp = wa.tile([P, KC * P], WDT, tag="wqp", bufs=2,
                              name=f"wqp_{p}")
                wkp = wa.tile([P, KC * P], WDT, tag="wkp", bufs=2,
                              name=f"wkp_{p}")
                nc.sync.dma_start(
                    wqp[:].rearrange("q (c n) -> q c n", n=P),
                    wq_r[:, :, p * P:(p + 1) * P])
                nc.scalar.dma_start(
                    wkp[:].rearrange("q (c n) -> q c n", n=P),
                    wk_r[:, :, p * P:(p + 1) * P])
                for half in range(2):
                    for w_sb, b_sb, dst in ((wqp, bqt, qT), (wkp, bkt, kT)):
                        ps = pp.tile([P, 512], F32, tag="ps")
                        for kc in range(KC):
                            nc.tensor.matmul(
                                ps[:],
                                lhsT=fr(w_sb[:, kc * P:(kc + 1) * P]),
                                rhs=fr(xT[:, kc * S + half * 512: kc * S + (half + 1) * 512]),
                                start=(kc == 0), stop=(kc == KC - 1))
                        # bias is per output partition (head-major row):
                        # fold it into the psum->sbuf copy
                        nc.vector.tensor_scalar_add(
                            dst[:, p * S + half * 512: p * S + (half + 1) * 512],
                            ps[:], b_sb[:, p:p + 1])

            # ---- V projection into v_aug (strided per-head placement)
            for s in range(SB):
                for cb, n_cols in ((0, 512), (1, 256)):
                    ps = pp.tile([P, n_cols], F32, tag="ps")
                    for kc in range(KC):
                        nc.tensor.matmul(
                            ps[:],
                            lhsT=fr(xT[:, kc * S + s * P: kc * S + (s + 1) * P]),
                            rhs=fr(wv[:, kc * D + cb * 512: kc * D + cb * 512 + n_cols]),
                            start=(kc == 0), stop=(kc == KC - 1))
                    nh = n_cols // DH  # heads in this column block
                    base = s * H * VW + cb * 8 * VW
                    dst3 = vaug[:, base: base + nh * VW].rearrange(
                        "p (n c) -> p n c", c=VW)[:, :, 0:DH]
                    src3 = ps[:, 0:n_cols].rearrange("p (n c) -> p n c", c=DH)
                    bv3 = bvb[:, cb * 512: cb * 512 + n_cols].rearrange(
                        "p (n c) -> p n c", c=DH)
                    nc.vector.tensor_add(dst3, src3, bv3)

        # ---- attention per head pair
        # score_pieces(j): q-ranges for the score matmuls / exp, chosen >=256
        # wide where possible (fp32r matmuls run 4x slower below N=256).
        # av_slices(j): q-ranges of AV matmuls, aligned to the two z psum
        # banks (the AV rhs reads the exp SBUF tile, so the boundaries are
        # independent of score_pieces).
        # qs_eff: the q-start of the computed strip for kblock j.  For j=3 and
        # j=7 the strip is widened 128 into the masked region so that every
        # score/AV matmul has free dim >= 256 (fp32r runs 4x slower below
        # that); the widened part is zeroed by the affine_select, so the AV
        # accumulation just adds zeros there.
        def qs_eff(j):
            return (j - 1) * P if j in (3, 7) else j * P

        def score_pieces(j):
            qs, L, out = qs_eff(j), S - qs_eff(j), []
            while L > 0:
                w = 512 if L >= 768 else (L if L <= 512 else L - 256)
                out.append((qs, qs + w))
                qs += w
                L -= w
            return out

        def av_slices(j):
            qs = qs_eff(j)
            sl = []
            if qs < 512:
                sl.append((qs, 512, 0))
            sl.append((max(qs, 512), 1024, 1))
            return sl

        for p in range(PAIRS):
            zps = [[None, None], [None, None]]  # [o][chunk]
            for o in range(2):
                for c in range(2):
                    zps[o][c] = pp.tile([P, 512], F32, tag="ps",
                                        name=f"zps_{p}_{o}_{c}")
            for j in range(SB):
                q0 = qs_eff(j)
                expt = [None, None]
                for o in range(2):
                    expt[o] = exp_pool.tile([P, S - q0], FR, tag="exp",
                                            name=f"exp_{p}_{j}_{o}")
                # scores + exp (interleave heads for PE row-group concurrency)
                for (qs, qe) in score_pieces(j):
                    sps = [None, None]
                    for o in range(2):
                        sps[o] = pp.tile([P, qe - qs], F32, tag="ps",
                                         name=f"sps_{p}_{j}_{qs}_{o}")
                        nc.tensor.matmul(
                            sps[o][:],
                            lhsT=fr(kT[o * DH:(o + 1) * DH, p * S + j * P: p * S + (j + 1) * P]),
                            rhs=fr(qT[o * DH:(o + 1) * DH, p * S + qs: p * S + qe]),
                            start=True, stop=True)
                    for o in range(2):
                        nc.scalar.activation(
                            expt[o][:, qs - q0: qe - q0], sps[o][:],
                            mybir.ActivationFunctionType.Exp, scale=0.125)
                # zero where q < k over the leading cols (diagonal block plus
                # any widened pre-diagonal region): keep iff
                # (q0 - j*128) + col - partition >= 0
                wz = j * P + P - q0
                for o in range(2):
                    nc.gpsimd.affine_select(
                        out=expt[o][:, 0:wz], in_=expt[o][:, 0:wz],
                        compare_op=mybir.AluOpType.is_ge,
                        fill=0.0, base=q0 - j * P,
                        pattern=[[1, wz]], channel_multiplier=-1)
                # AV accumulation (+ denominator in partitions 64..127)
                for o in range(2):
                    n = 2 * p + o
                    for (qs, qe, c) in av_slices(j):
                        nc.tensor.matmul(
                            zps[o][c][:, qs - c * 512: qe - c * 512],
                            lhsT=fr(vaug[:, j * H * VW + n * VW: j * H * VW + (n + 1) * VW]),
                            rhs=fr(expt[o][:, qs - q0: qe - q0]),
                            start=(j == 0),
                            stop=(j == 3 if c == 0 else j == 7))
                # chunk 0 finishes at j==3: normalize early to free the bank
                if j == 3 or j == 7:
                    c = 0 if j == 3 else 1
                    for o in range(2):
                        n = 2 * p + o
                        rec = rec_pool.tile([DH, 512], F32, tag="rec")
                        nc.vector.reciprocal(rec[:], zps[o][c][DH:P, 0:512])
                        nc.vector.tensor_mul(
                            zT[o * DH:(o + 1) * DH, p * S + c * 512: p * S + (c + 1) * 512],
                            zps[o][c][0:DH, 0:512], rec[:])

        # ---- output projection: out[s*128.., m] = zT.T @ W_O + b_O
        with tc.tile_pool(name="wc", bufs=1) as wc:
            wo = wc.tile([P, KC * D], FR, tag="wo")
            for kc in range(KC):
                nc.sync.dma_start(wo[:, kc * D:(kc + 1) * D],
                                  ins["wo"][kc * P:(kc + 1) * P, :])
            for s in range(SB):
                outb = out_pool.tile([P, D], F32, tag="outb")
                for cb, n_cols in ((0, 512), (1, 256)):
                    ps = pp.tile([P, n_cols], F32, tag="ps")
                    for p in range(PAIRS):
                        nc.tensor.matmul(
                            ps[:],
                            lhsT=fr(zT[:, p * S + s * P: p * S + (s + 1) * P]),
                            rhs=fr(wo[:, p * D + cb * 512: p * D + cb * 512 + n_cols]),
                            start=(p == 0), stop=(p == PAIRS - 1))
                    nc.vector.tensor_add(outb[:, cb * 512: cb * 512 + n_cols],
                                         ps[:],
                                         bob[:, cb * 512: cb * 512 + n_cols])
                    nc.sync.dma_start(
                        out_ap[s * P:(s + 1) * P, cb * 512: cb * 512 + n_cols],
                        outb[:, cb * 512: cb * 512 + n_cols])


_CACHED = {}


def build_program(reps=1):
    if reps in _CACHED:
        return _CACHED[reps]
    nc = bacc.Bacc("TRN2", target_bir_lowering=False, debug=False)
    ins = {
        "xT": nc.dram_tensor("xT", [D, S], FR, kind="ExternalInput").ap(),
        "wq": nc.dram_tensor("wq", [D, D], WDT, kind="ExternalInput").ap(),
        "wk": nc.dram_tensor("wk", [D, D], WDT, kind="ExternalInput").ap(),
        "wv": nc.dram_tensor("wv", [D, D], FR, kind="ExternalInput").ap(),
        "wo": nc.dram_tensor("wo", [D, D], FR, kind="ExternalInput").ap(),
        "bqt": nc.dram_tensor("bqt", [D, 1], F32, kind="ExternalInput").ap(),
        "bkt": nc.dram_tensor("bkt", [D, 1], F32, kind="ExternalInput").ap(),
        "bvb": nc.dram_tensor("bvb", [P, D], F32, kind="ExternalInput").ap(),
        "bob": nc.dram_tensor("bob", [P, D], F32, kind="ExternalInput").ap(),
    }
    out = nc.dram_tensor("out", [S, D], F32, kind="ExternalOutput").ap()
    with tile.TileContext(nc) as tc:
        for _ in range(reps):
            attention_kernel(tc, out, ins)
    nc.compile()
    _CACHED[reps] = nc
    return nc


def make_in_maps(normalized_resid_pre, W_Q, W_K, W_V, W_O, b_Q, b_K, b_V, b_O):
    x = np.asarray(normalized_resid_pre, np.float32)
    wdt = np.float32
    if BF16_LHS:
        import ml_dtypes
        wdt = ml_dtypes.bfloat16
    wq_m = np.ascontiguousarray(
        np.asarray(W_Q, np.float32).transpose(1, 0, 2).reshape(D, D).astype(wdt))
    wk_m = np.ascontiguousarray(
        np.asarray(W_K, np.float32).transpose(1, 0, 2).reshape(D, D).astype(wdt))
    wv_m = np.ascontiguousarray(
        np.asarray(W_V, np.float32).transpose(1, 0, 2).reshape(D, D))
    wo_m = np.ascontiguousarray(np.asarray(W_O, np.float32).reshape(D, D))
    bq_m = np.asarray(b_Q, np.float32).reshape(D, 1)
    bk_m = np.asarray(b_K, np.float32).reshape(D, 1)
    bv_m = np.ascontiguousarray(np.broadcast_to(
        np.asarray(b_V, np.float32).reshape(1, D), (P, D)))
    bo_m = np.ascontiguousarray(np.broadcast_to(
        np.asarray(b_O, np.float32).reshape(1, D), (P, D)))
    in_maps = []
    for b in range(N_CORES):
        in_maps.append({
            "xT": np.ascontiguousarray(x[b].T),
            "wq": wq_m, "wk": wk_m, "wv": wv_m, "wo": wo_m,
            "bqt": bq_m, "bkt": bk_m, "bvb": bv_m, "bob": bo_m,
        })
    return in_maps


def kernel(**inputs):
    nc = build_program()
    in_maps = make_in_maps(**inputs)
    res = bass_utils.run_bass_kernel_spmd(nc, in_maps, list(range(N_CORES)))
    return np.stack([r["out"] for r in res.results])


# revision 27
# speedup vs baseline: 1.2298x; 1.2298x over previous
"""Causal multi-head attention on 8 Trainium2 NeuronCores.

Problem: x[8,1024,768], 12 heads of d_head=64, causal softmax attention,
output projection. Sharding: data-parallel over batch (8 batch elements ==
8 cores), no collectives.

Per-core layout strategy (zero on-device transposes):
  - host passes xT [768,1024] (d_model on partitions)
  - qT/kT [768,1024] head-major rows  (d_head on partitions, seq on free)
  - scoresT[k, q] = kT_blk.T @ qT     (k on partitions, q on free)
  - v_aug [1024, 12*128]: per head 64 v columns + 64 ones columns; the AV
    matmul zT_psum = v_aug_blk.T @ exp(scoresT) then yields the softmax
    denominator (replicated) in psum partitions 64..127 for free
  - normalize with DVE reciprocal + tensor_mul (one PSUM operand)
  - out = zT.T @ W_O accumulated over head-pair chunks
Causal structure: only the lower-triangular (k <= q) blocks are computed;
the diagonal 128x128 block is zeroed above the diagonal post-exp via
affine_select. exp is computed without max subtraction (scores are O(1)
by construction, exp(-1e5) == 0 in fp32 matches the reference's masking).
All matmuls run as float32r (full PE rate) accumulating fp32 in PSUM.
"""

from contextlib import ExitStack

import numpy as np

import concourse.bass as bass
import concourse.mybir as mybir
import concourse.tile as tile
from concourse import bacc, bass_utils

F32 = mybir.dt.float32
FR = mybir.dt.float32r
BF = mybir.dt.bfloat16
import os
# BASS_BF16=1: all matmul operands in bf16 (mixing bf16/fp32r is illegal on
# TRN2, so it is all-or-nothing).  PSUM accumulation stays fp32 either way.
BF16 = os.environ.get("BASS_BF16", "0") == "1"
MDT = BF if BF16 else FR           # dtype of every matmul operand tile

S = 1024        # seq len
D = 768         # d_model
H = 12          # heads
DH = 64         # d_head
P = 128         # partitions
KC = D // P     # 6 k-chunks of d_model
SB = S // P     # 8 seq blocks
PAIRS = H // 2  # 6 head pairs
VW = 2 * DH     # 128: v cols + ones cols per head in v_aug
N_CORES = 8


def fr(ap):
    return ap


def attention_kernel(tc, out_ap, ins):
    nc = tc.nc
    with ExitStack() as ctx:
        cpool = ctx.enter_context(tc.tile_pool(name="consts", bufs=1))
        big = ctx.enter_context(tc.tile_pool(name="big", bufs=1))
        exp_pool = ctx.enter_context(tc.tile_pool(name="exp", bufs=3))
        rec_pool = ctx.enter_context(tc.tile_pool(name="rec", bufs=2))
        out_pool = ctx.enter_context(tc.tile_pool(name="outb", bufs=3))
        pp = ctx.enter_context(tc.tile_pool(name="ps", bufs=8, space="PSUM"))

        # ---- big persistent tiles.  zT reuses xT's storage: xT is fully
        # consumed by the projections before any zT column is written (the
        # scheduler enforces the WAR ordering via subtile deps).
        xT = big.tile([P, KC * S], MDT, tag="xT")          # xT[kc]: cols kc*1024
        qT = big.tile([P, PAIRS * S], MDT, tag="qT")       # pair p: cols p*1024+q
        kT = big.tile([P, PAIRS * S], MDT, tag="kT")
        vaug = big.tile([P, SB * H * VW], MDT, tag="vaug")  # blk j: j*1536 + n*128
        zT = xT
        xv = xT

        # xT loads issued on the ACT sequencer so they don't queue behind the
        # W loads on SP (each DMA costs ~0.6us of issue time per sequencer)
        for kc in range(KC):
            nc.scalar.dma_start(xT[:, kc * S:(kc + 1) * S],
                                ins["xT"][kc * P:(kc + 1) * P, :])

        # ---- constants / small tiles (issued on the Pool sequencer)
        bqt = cpool.tile([P, PAIRS], F32, tag="bqt")   # b_Q per head-major row
        bkt = cpool.tile([P, PAIRS], F32, tag="bkt")
        bvb = cpool.tile([P, D], F32, tag="bvb")       # b_V broadcast to 128 rows
        bob = cpool.tile([P, D], F32, tag="bob")       # b_O broadcast to 128 rows
        nc.gpsimd.dma_start(bqt[:], ins["bqt"].rearrange("(c p) x -> p (c x)", p=P))
        nc.gpsimd.dma_start(bkt[:], ins["bkt"].rearrange("(c p) x -> p (c x)", p=P))
        nc.gpsimd.dma_start(bvb[:], ins["bvb"][:])
        nc.gpsimd.dma_start(bob[:], ins["bob"][:])

        # ones columns of v_aug (cols 64..127 of each head block), filled by
        # DVE copies (f32 -> fp32r rounding) from a memset source tile
        ones_f = cpool.tile([P, D], F32, tag="ones_f")
        nc.gpsimd.memset(ones_f[:], 1.0)
        ones_f3 = ones_f[:].rearrange("p (x c) -> p x c", c=DH)  # [128, 12, 64]
        for s in range(SB):
            blk = vaug[:, s * H * VW:(s + 1) * H * VW]
            va3 = blk.rearrange("p (x c) -> p x c", c=VW)
            nc.vector.tensor_copy(va3[:, :, DH:VW], ones_f3)

        # per-pair column slices of W_Q/W_K, streamed: [768, 128] -> [128, 6*128]
        wq_r = ins["wq"].rearrange("(c p) n -> p c n", p=P)
        wk_r = ins["wk"].rearrange("(c p) n -> p c n", p=P)

        with tc.tile_pool(name="wa", bufs=1) as wa:
            wv = wa.tile([P, KC * D], MDT, tag="wv")
            for kc in range(KC):
                nc.gpsimd.dma_start(wv[:, kc * D:(kc + 1) * D],
                                    ins["wv"][kc * P:(kc + 1) * P, :])

            # ---- Q/K projections: qT[pair rows, q] = W[:, pair].T @ xT
            for p in range(PAIRS):
                wqp = wa.tile([P, KC * P], MDT, tag="wqp", bufs=2,
                              name=f"wqp_{p}")
                wkp = wa.tile([P, KC * P], MDT, tag="wkp", bufs=2,
                              name=f"wkp_{p}")
                nc.sync.dma_start(
                    wqp[:].rearrange("q (c n) -> q c n", n=P),
                    wq_r[:, :, p * P:(p + 1) * P])
                nc.scalar.dma_start(
                    wkp[:].rearrange("q (c n) -> q c n", n=P),
                    wk_r[:, :, p * P:(p + 1) * P])
                for half in range(2):
                    for w_sb, b_sb, dst in ((wqp, bqt, qT), (wkp, bkt, kT)):
                        ps = pp.tile([P, 512], F32, tag="ps")
                        for kc in range(KC):
                            nc.tensor.matmul(
                                ps[:],
                                lhsT=fr(w_sb[:, kc * P:(kc + 1) * P]),
                                rhs=fr(xT[:, kc * S + half * 512: kc * S + (half + 1) * 512]),
                                start=(kc == 0), stop=(kc == KC - 1))
                        # bias is per output partition (head-major row):
                        # fold it into the psum->sbuf copy
                        nc.vector.tensor_scalar_add(
                            dst[:, p * S + half * 512: p * S + (half + 1) * 512],
                            ps[:], b_sb[:, p:p + 1])

            # ---- V projection into v_aug (strided per-head placement)
            for s in range(SB):
                for cb, n_cols in ((0, 512), (1, 256)):
                    ps = pp.tile([P, n_cols], F32, tag="ps")
                    for kc in range(KC):
                        nc.tensor.matmul(
                            ps[:],
                            lhsT=fr(xv[:, kc * S + s * P: kc * S + (s + 1) * P]),
                            rhs=fr(wv[:, kc * D + cb * 512: kc * D + cb * 512 + n_cols]),
                            start=(kc == 0), stop=(kc == KC - 1))
                    nh = n_cols // DH  # heads in this column block
                    base = s * H * VW + cb * 8 * VW
                    dst3 = vaug[:, base: base + nh * VW].rearrange(
                        "p (n c) -> p n c", c=VW)[:, :, 0:DH]
                    src3 = ps[:, 0:n_cols].rearrange("p (n c) -> p n c", c=DH)
                    bv3 = bvb[:, cb * 512: cb * 512 + n_cols].rearrange(
                        "p (n c) -> p n c", c=DH)
                    nc.vector.tensor_add(dst3, src3, bv3)

        # ---- attention per head pair
        # score_pieces(j): q-ranges for the score matmuls / exp, chosen >=256
        # wide where possible (fp32r matmuls run 4x slower below N=256).
        # av_slices(j): q-ranges of AV matmuls, aligned to the two z psum
        # banks (the AV rhs reads the exp SBUF tile, so the boundaries are
        # independent of score_pieces).
        # qs_eff: the q-start of the computed strip for kblock j.  For j=3 and
        # j=7 the strip is widened 128 into the masked region so that every
        # score/AV matmul has free dim >= 256 (fp32r runs 4x slower below
        # that); the widened part is zeroed by the affine_select, so the AV
        # accumulation just adds zeros there.
        def qs_eff(j):
            return (j - 1) * P if j in (3, 7) else j * P

        def score_pieces(j):
            qs, L, out = qs_eff(j), S - qs_eff(j), []
            while L > 0:
                w = 512 if L >= 768 else (L if L <= 512 else L - 256)
                out.append((qs, qs + w))
                qs += w
                L -= w
            return out

        def av_slices(j):
            qs = qs_eff(j)
            sl = []
            if qs < 512:
                sl.append((qs, 512, 0))
            sl.append((max(qs, 512), 1024, 1))
            return sl

        for p in range(PAIRS):
            zps = [[None, None], [None, None]]  # [o][chunk]
            for o in range(2):
                for c in range(2):
                    zps[o][c] = pp.tile([P, 512], F32, tag="ps",
                                        name=f"zps_{p}_{o}_{c}")
            for j in range(SB):
                q0 = qs_eff(j)
                expt = [None, None]
                for o in range(2):
                    expt[o] = exp_pool.tile([P, S - q0], MDT, tag="exp",
                                            name=f"exp_{p}_{j}_{o}")
                # scores + exp (interleave heads for PE row-group concurrency)
                for (qs, qe) in score_pieces(j):
                    sps = [None, None]
                    for o in range(2):
                        sps[o] = pp.tile([P, qe - qs], F32, tag="ps",
                                         name=f"sps_{p}_{j}_{qs}_{o}")
                        nc.tensor.matmul(
                            sps[o][:],
                            lhsT=fr(kT[o * DH:(o + 1) * DH, p * S + j * P: p * S + (j + 1) * P]),
                            rhs=fr(qT[o * DH:(o + 1) * DH, p * S + qs: p * S + qe]),
                            start=True, stop=True)
                    for o in range(2):
                        nc.scalar.activation(
                            expt[o][:, qs - q0: qe - q0], sps[o][:],
                            mybir.ActivationFunctionType.Exp, scale=0.125)
                # zero where q < k over the leading cols (diagonal block plus
                # any widened pre-diagonal region): keep iff
                # (q0 - j*128) + col - partition >= 0
                wz = j * P + P - q0
                for o in range(2):
                    nc.gpsimd.affine_select(
                        out=expt[o][:, 0:wz], in_=expt[o][:, 0:wz],
                        compare_op=mybir.AluOpType.is_ge,
                        fill=0.0, base=q0 - j * P,
                        pattern=[[1, wz]], channel_multiplier=-1)
                # AV accumulation (+ denominator in partitions 64..127)
                for o in range(2):
                    n = 2 * p + o
                    for (qs, qe, c) in av_slices(j):
                        nc.tensor.matmul(
                            zps[o][c][:, qs - c * 512: qe - c * 512],
                            lhsT=fr(vaug[:, j * H * VW + n * VW: j * H * VW + (n + 1) * VW]),
                            rhs=fr(expt[o][:, qs - q0: qe - q0]),
                            start=(j == 0),
                            stop=(j == 3 if c == 0 else j == 7))
                # chunk 0 finishes at j==3: normalize early to free the bank
                if j == 3 or j == 7:
                    c = 0 if j == 3 else 1
                    for o in range(2):
                        n = 2 * p + o
                        rec = rec_pool.tile([DH, 512], F32, tag="rec")
                        nc.vector.reciprocal(rec[:], zps[o][c][DH:P, 0:512])
                        nc.vector.tensor_mul(
                            zT[o * DH:(o + 1) * DH, p * S + c * 512: p * S + (c + 1) * 512],
                            zps[o][c][0:DH, 0:512], rec[:])

        # ---- output projection: out[s*128.., m] = zT.T @ W_O + b_O
        with tc.tile_pool(name="wc", bufs=1) as wc:
            wo = wc.tile([P, KC * D], MDT, tag="wo")
            for kc in range(KC):
                nc.sync.dma_start(wo[:, kc * D:(kc + 1) * D],
                                  ins["wo"][kc * P:(kc + 1) * P, :])
            for s in range(SB):
                outb = out_pool.tile([P, D], F32, tag="outb")
                for cb, n_cols in ((0, 512), (1, 256)):
                    ps = pp.tile([P, n_cols], F32, tag="ps")
                    for p in range(PAIRS):
                        nc.tensor.matmul(
                            ps[:],
                            lhsT=fr(zT[:, p * S + s * P: p * S + (s + 1) * P]),
                            rhs=fr(wo[:, p * D + cb * 512: p * D + cb * 512 + n_cols]),
                            start=(p == 0), stop=(p == PAIRS - 1))
                    nc.vector.tensor_add(outb[:, cb * 512: cb * 512 + n_cols],
                                         ps[:],
                                         bob[:, cb * 512: cb * 512 + n_cols])
                    nc.sync.dma_start(
                        out_ap[s * P:(s + 1) * P, cb * 512: cb * 512 + n_cols],
                        outb[:, cb * 512: cb * 512 + n_cols])


_CACHED = {}


def build_program(reps=1):
    if reps in _CACHED:
        return _CACHED[reps]
    nc = bacc.Bacc("TRN2", target_bir_lowering=False, debug=False)
    ins = {
        "xT": nc.dram_tensor("xT", [D, S], MDT, kind="ExternalInput").ap(),
        "wq": nc.dram_tensor("wq", [D, D], MDT, kind="ExternalInput").ap(),
        "wk": nc.dram_tensor("wk", [D, D], MDT, kind="ExternalInput").ap(),
        "wv": nc.dram_tensor("wv", [D, D], MDT, kind="ExternalInput").ap(),
        "wo": nc.dram_tensor("wo", [D, D], MDT, kind="ExternalInput").ap(),
        "bqt": nc.dram_tensor("bqt", [D, 1], F32, kind="ExternalInput").ap(),
        "bkt": nc.dram_tensor("bkt", [D, 1], F32, kind="ExternalInput").ap(),
        "bvb": nc.dram_tensor("bvb", [P, D], F32, kind="ExternalInput").ap(),
        "bob": nc.dram_tensor("bob", [P, D], F32, kind="ExternalInput").ap(),
    }
    out = nc.dram_tensor("out", [S, D], F32, kind="ExternalOutput").ap()
    with tile.TileContext(nc) as tc:
        for _ in range(reps):
            attention_kernel(tc, out, ins)
    nc.compile()
    _CACHED[reps] = nc
    return nc


def make_in_maps(normalized_resid_pre, W_Q, W_K, W_V, W_O, b_Q, b_K, b_V, b_O):
    x = np.asarray(normalized_resid_pre, np.float32)
    wdt = np.float32
    if BF16:
        import ml_dtypes
        wdt = ml_dtypes.bfloat16
    wq_m = np.ascontiguousarray(
        np.asarray(W_Q, np.float32).transpose(1, 0, 2).reshape(D, D).astype(wdt))
    wk_m = np.ascontiguousarray(
        np.asarray(W_K, np.float32).transpose(1, 0, 2).reshape(D, D).astype(wdt))
    wv_m = np.ascontiguousarray(
        np.asarray(W_V, np.float32).transpose(1, 0, 2).reshape(D, D).astype(wdt))
    wo_m = np.ascontiguousarray(
        np.asarray(W_O, np.float32).reshape(D, D).astype(wdt))
    bq_m = np.asarray(b_Q, np.float32).reshape(D, 1)
    bk_m = np.asarray(b_K, np.float32).reshape(D, 1)
    bv_m = np.ascontiguousarray(np.broadcast_to(
        np.asarray(b_V, np.float32).reshape(1, D), (P, D)))
    bo_m = np.ascontiguousarray(np.broadcast_to(
        np.asarray(b_O, np.float32).reshape(1, D), (P, D)))
    in_maps = []
    for b in range(N_CORES):
        in_maps.append({
            "xT": np.ascontiguousarray(x[b].T).astype(wdt),
            "wq": wq_m, "wk": wk_m, "wv": wv_m, "wo": wo_m,
            "bqt": bq_m, "bkt": bk_m, "bvb": bv_m, "bob": bo_m,
        })
    return in_maps


def kernel(**inputs):
    nc = build_program()
    in_maps = make_in_maps(**inputs)
    res = bass_utils.run_bass_kernel_spmd(nc, in_maps, list(range(N_CORES)))
    return np.stack([r["out"] for r in res.results])
